# revision 48
# baseline (speedup 1.0000x reference)
"""Trainium2 Bass kernel for nn_CausalLinearSelfAttention_30013231464545.

Math: the reference cumsums the [B,T,H,D,M] kv tensor over axis=-2 (the D
axis), so the module collapses to
  out[b,t,h,m] = v[b,t,h,m] * s[b,t,h] / denom[b,t,h]
  denom = sum_d qf * cumsum_T(kf)   (causal running key sum)
  s     = sum_d qf * cumsum_D(kf)   (per-timestep D-prefix sum)
  qf = elu(q)+1 = min(exp(q),1) + relu(q),  kf likewise.

Sharding: core c -> (b = c//4, heads 2*(c%4)..2*(c%4)+1), slice [T=2048, 128].

v2 design (f16 end-to-end on chip, engine-balanced):
- Host casts to f16 and uploads: qT [128=(h,d), 2048=t] pre-transposed, and a
  pre-tiled kv blob [128=t-in-tile, (16 tiles x (k128|v128))]. Output is a
  packed [128, (16 tiles x 128)] f16 blob unpacked on host. All DMAs are
  128 descriptors x >=1KB (vs 512B in v1), input traffic halves.
- Feature maps: tensor_scalar relu/min clamps run in DVE 4x mode, one ACT Exp
  pass per tensor, one 2x tensor_tensor add.
- cumsum over T is computed TRANSPOSED on the PE: per t-tile, stationary =
  kf_j [t', (h,d)], moving = [triu128 | ones] so one matmul yields both the
  within-tile cumsum lcsT[(h,d), t] and the tile colsum (129th column).
  Cross-tile carries: tiny DVE prefix scan over colsums; the carry is added
  per-partition via the scalar slot of the denom-product scalar_tensor_tensor.
- cumsum over D: kfT via per-tile DMA transposes, then one block-triangular
  matmul per chunk -> csD_T in PSUM (replaces the slow DVE segmented scan).
- s/denom dots: DVE multiplies into a packed prods tile, then PE reduces over
  d-partitions per (tile, head) via a 0/1 selector matmul into sd_ps [8, 256];
  the cross-tile carry enters as bases.qf matmuls accumulated into the same
  PSUM tile.
- scale = s * recip(denom) (fast approx), PE-transposed back to [t, (j,h)],
  broadcast-multiplied into v, stored per chunk.
- Scheduling: engines dispatch in program order, so emission is two-phase
  (all features/cumsums, then dots/scale/output) and phase B is software-
  pipelined one chunk deep so the in-order DVE never head-of-line stalls
  on the PE's sd reduction.
"""

import numpy as np
import sys

sys.path.insert(0, "/opt/trn_rl_repo")

B, T, H, D = 2, 2048, 8, 64
P = 128          # partitions
HPC = 2          # heads per core
C = HPC * D      # 128: (h,d) width per core
NT = T // P      # 16 t-tiles
NCH = 4          # pipeline chunks
JT = NT // NCH   # 4 tiles per chunk
KVW = 2 * C      # 256: k|v columns per tile in the kv blob

DN_ON_POOL = False  # gpsimd cannot read PSUM on trn2 (BIR verifier)
DEBUG_SD = False    # dump per-chunk sd [8,256] (s|denom reductions) to DRAM

_CACHE = {}


def _build_nc():
    import concourse.bass as bass
    import concourse.bacc as bacc
    import concourse.mybir as mybir
    from concourse import tile

    dt = mybir.dt
    f32 = dt.float32
    f16 = dt.float16
    Alu = mybir.AluOpType
    Act = mybir.ActivationFunctionType

    nc = bacc.Bacc(None)

    qt_d = nc.declare_dram_parameter("qt", [P, T], f16, isOutput=False)
    kv_d = nc.declare_dram_parameter("kv", [P, NT * KVW], f16, isOutput=False)
    o_d = nc.declare_dram_parameter("o", [P, NT * C], f16, isOutput=True)
    if DEBUG_SD:
        dbg_d = nc.declare_dram_parameter("dbg", [8 * NCH, 256], f32,
                                          isOutput=True)

    # ---- constants, packed into one DRAM tensor [128, 296] ----
    # cols 0:128   tri: triu128 (within-tile cumsum; col 127 = colsum)
    # cols 128:256 btri: per-head upper-tri 64 blocks (cumsum over D)
    # cols 256:288 sel: (tile,head) selector columns for the d-reduce
    # cols 288:296 id8 (rows 0:8): identity for the scale transpose
    CW = 128 + 128 + 32 + 512
    cst = np.zeros((P, CW), dtype=np.float16)
    cst[:, 0:128] = np.triu(np.ones((P, P)))
    cst[:, 128:256] = np.kron(np.eye(2), np.triu(np.ones((64, 64))))
    for jl in range(JT):
        for i in range(8):
            # sel[p, jl*8+i] = 1 iff i == jl*2 + head(p)
            h = i - 2 * jl
            if h in (0, 1):
                cst[h * 64:(h + 1) * 64, 256 + jl * 8 + i] = 1.0
    cst[0:8, 288:296] = np.eye(8)
    cst_d = nc.inline_tensor(cst, name="cst_const")

    with tile.TileContext(nc) as tc:
        with (
            tc.tile_pool(name="const", bufs=1) as cpool,
            tc.tile_pool(name="io", bufs=1) as io,
            tc.tile_pool(name="ot", bufs=2) as otp,
            tc.tile_pool(name="wk", bufs=4) as wk,
            tc.tile_pool(name="sm", bufs=1) as sm,
            tc.tile_pool(name="pl", bufs=3, space="PSUM") as pl,
            tc.tile_pool(name="pc", bufs=1, space="PSUM") as pcd,
            tc.tile_pool(name="pd", bufs=2, space="PSUM") as pd,
            tc.tile_pool(name="pt", bufs=2, space="PSUM") as pt,
        ):
            qtw = io.tile([P, T], f16, tag="qt")
            kvw = io.tile([P, NT * KVW], f16, tag="kv")

            def ld(eng, dst, src, c0, c1, w):
                eng.dma_start(dst[:, c0 * w:(c1 + 1) * w],
                              src[:, c0 * w:(c1 + 1) * w])

            QW = JT * C        # 512 qT cols per chunk
            VW = JT * KVW      # 1024 kv cols per chunk
            # warm-up: a tiny sync-ring DMA plus an ACT->DVE handoff so the
            # first real crossings don't pay the cold semaphore latency
            warm = cpool.tile([1, 16], f16, tag="warm")
            nc.sync.dma_start(warm[:, 0:8], cst_d[0:1, 0:8])
            nc.scalar.activation(warm[:, 8:12], warm[:, 0:4], Act.Exp)
            nc.vector.tensor_scalar(warm[:, 12:16], warm[:, 8:12], 0.0, None,
                                    op0=Alu.max)
            ld(nc.sync, kvw, kv_d, 0, 0, VW)
            ld(nc.scalar, qtw, qt_d, 0, 0, QW)
            ld(nc.sync, kvw, kv_d, 1, 1, VW)
            ld(nc.scalar, qtw, qt_d, 1, 1, QW)
            ld(nc.sync, kvw, kv_d, 2, 3, VW)
            ld(nc.scalar, qtw, qt_d, 2, 3, QW)

            cw = cpool.tile([P, CW], f16, tag="cst")
            nc.scalar.dma_start(cw[:], cst_d[:])
            tri = cw[:, 0:128]
            btri = cw[:, 128:256]
            sel = cw[:, 256:288]
            id8 = cw[0:8, 288:296]

            kfw = sm.tile([P, T], f16, tag="kfw")        # feature(k), full
            kftw = sm.tile([P, T], f16, tag="kftw")      # kf transposed
            sctc = sm.tile([P, 8 * NCH], f16, tag="sctc")  # scale, SBUF copy
            cs_sb = sm.tile([P, NT], f16, tag="cs")      # tile colsums
            btmp = sm.tile([P, NT], f16, tag="btmp")     # inclusive prefix
            bases = sm.tile([P, NT], f16, tag="bases")   # exclusive prefix
            ones = sm.tile([P, NT], f16, tag="ones")
            nc.gpsimd.memset(ones[:], 1.0)

            qfts, lcss, csds = [], [], []

            # ---------- phase A: features, cumsums, carries ----------
            for ch in range(NCH):
                j0 = ch * JT
                kview = kvw[:, ch * VW:(ch + 1) * VW].rearrange(
                    "p (j c) -> p j c", c=KVW)[:, :, 0:C]
                qsl = qtw[:, ch * QW:(ch + 1) * QW]

                # feature maps: f = min(exp(x),1) + relu(x)
                kf = kfw[:, ch * QW:(ch + 1) * QW]
                rk = wk.tile([P, QW], f16, tag="rk")
                nc.vector.tensor_scalar(
                    rk[:].rearrange("p (j c) -> p j c", c=C),
                    kview, 0.0, None, op0=Alu.max)
                ek = wk.tile([P, QW], f16, tag="ek")
                nc.scalar.activation(
                    ek[:].rearrange("p (j c) -> p j c", c=C),
                    kview, Act.Exp)
                emk = wk.tile([P, QW], f16, tag="emk")
                nc.vector.tensor_scalar(emk[:], ek[:], 1.0, None, op0=Alu.min)
                nc.vector.tensor_tensor(kf, emk[:], rk[:], op=Alu.add)

                rq = wk.tile([P, QW], f16, tag="rq")
                nc.vector.tensor_scalar(rq[:], qsl, 0.0, None, op0=Alu.max)
                eq = wk.tile([P, QW], f16, tag="eq")
                nc.scalar.activation(eq[:], qsl, Act.Exp)
                emq = wk.tile([P, QW], f16, tag="emq")
                nc.vector.tensor_scalar(emq[:], eq[:], 1.0, None, op0=Alu.min)
                qft = wk.tile([P, QW], f16, tag="qft")
                nc.vector.tensor_tensor(qft[:], emq[:], rq[:], op=Alu.add)
                qfts.append(qft)

                # transposed within-tile cumsum over T (PE); the cumsum's
                # last column doubles as the tile colsum
                lcs = pl.tile([P, JT * C], f32, tag="lcs")
                for jl in range(JT):
                    nc.tensor.matmul(
                        lcs[:, jl * C:(jl + 1) * C],
                        kfw[:, ch * QW + jl * C: ch * QW + (jl + 1) * C],
                        tri, start=True, stop=True,
                    )
                lcss.append(lcs)

                # kfT via batched per-block DMA transpose, then csD (PE)
                kft = kftw[:, ch * QW:(ch + 1) * QW]
                nc.sync.dma_start_transpose(
                    kft.rearrange("p (j t) -> p j t", t=P),
                    kfw[:, ch * QW:(ch + 1) * QW])
                csd = pcd.tile([P, QW], f32, tag="csd")
                nc.tensor.matmul(csd[:], btri, kft, start=True, stop=True)
                csds.append(csd)

            # ---------- phase B: dots, reduce, scale, output ----------
            # Software-pipelined one chunk deep: chunk ch+1's products are
            # emitted before chunk ch's scale/output so the in-order DVE
            # never head-of-line stalls on the PE's sd reduction.
            sds = [None] * NCH

            def b_front(ch):
                j0 = ch * JT
                qft, lcs, csd = qfts[ch], lcss[ch], csds[ch]

                # colsum extraction + cross-tile carry prefix
                nc.scalar.copy(
                    cs_sb[:, j0:j0 + JT].rearrange("p (j o) -> p j o", o=1),
                    lcs[:].rearrange("p (j c) -> p j c", c=C)[:, :, 127:128],
                )
                w = j0 + JT
                nc.vector.tensor_tensor_scan(
                    btmp[:, 0:w], ones[:, 0:w], cs_sb[:, 0:w], 0.0,
                    op0=Alu.mult, op1=Alu.add,
                )
                nc.vector.tensor_tensor(
                    bases[:, j0:w], btmp[:, j0:w], cs_sb[:, j0:w],
                    op=Alu.subtract,
                )

                # copy cumsums PSUM -> SBUF f16 on the idle scalar engine
                # so both product multiplies run in DVE 2x mode
                lcsb = wk.tile([P, QW], f16, tag="lcsb")
                nc.scalar.copy(lcsb[:], lcs[:])
                csdb = wk.tile([P, QW], f16, tag="csdb")
                nc.scalar.copy(csdb[:], csd[:])
                lsrc, csrc = lcsb[:], csdb[:]

                # prods = [s-prod | dn-prod] per tile; the cross-tile carry
                # enters below as extra PE matmuls on the dn columns
                prods = wk.tile([P, JT * 256], f16, tag="prods")
                pview = prods[:].rearrange("p (j c) -> p j c", c=256)
                nc.vector.tensor_tensor(
                    pview[:, :, 128:256],
                    lsrc.rearrange("p (j c) -> p j c", c=C),
                    qft[:].rearrange("p (j c) -> p j c", c=C),
                    op=Alu.mult,
                )
                nc.vector.tensor_tensor(
                    pview[:, :, 0:128],
                    csrc.rearrange("p (j c) -> p j c", c=C),
                    qft[:].rearrange("p (j c) -> p j c", c=C),
                    op=Alu.mult,
                )
                # basesM[p, (jl,i)] = bases[p, j0+jl] * sel[p, (jl,i)]
                basm = wk.tile([P, JT * 8], f16, tag="basm")
                nc.vector.tensor_tensor(
                    basm[:].rearrange("p (j i) -> p j i", i=8),
                    bases[:, j0:j0 + JT].rearrange(
                        "p (j o) -> p j o", o=1).broadcast_to([P, JT, 8]),
                    sel.rearrange("p (j i) -> p j i", i=8),
                    op=Alu.mult,
                )

                # d-reduce per (tile, head) on PE -> sd [8, 256]
                sd = pd.tile([8, 256], f32, tag="sd")
                for jl in range(JT):
                    nc.tensor.matmul(
                        sd[:],
                        sel[:, jl * 8:(jl + 1) * 8],
                        prods[:, jl * 256:(jl + 1) * 256],
                        start=(jl == 0), stop=False,
                    )
                for jl in range(JT):
                    nc.tensor.matmul(
                        sd[:, 128:256],
                        basm[:, jl * 8:(jl + 1) * 8],
                        qft[:, jl * C:(jl + 1) * C],
                        start=False, stop=(jl == JT - 1),
                    )
                sds[ch] = sd

            def b_back(ch):
                sd = sds[ch]
                vview = kvw[:, ch * VW:(ch + 1) * VW].rearrange(
                    "p (j c) -> p j c", c=KVW)[:, :, C:KVW]
                if DEBUG_SD:
                    sdc = sm.tile([8, 256], f32, tag=f"sdc{ch}")
                    nc.scalar.copy(sdc[:], sd[:])
                    nc.sync.dma_start(dbg_d[ch * 8:(ch + 1) * 8, :], sdc[:])

                # scale = s / denom, transposed back to [t, (j,h)]
                rec = sm.tile([8, P], f32, tag=f"rec{ch}")
                nc.vector.reciprocal_approx_fast(rec[:], sd[:, 128:256])
                ssm = sm.tile([8, P], f16, tag=f"ssm{ch}")
                nc.vector.tensor_tensor(ssm[:], sd[:, 0:128], rec[:],
                                        op=Alu.mult)
                sct = pt.tile([P, 8], f16, tag="sct")
                nc.tensor.transpose(sct[:], ssm[:], id8)

                # out = v * scale (broadcast over each head's 64 columns)
                ot = otp.tile([P, JT * C], f16, tag="o")
                sctb = sct[:].rearrange(
                    "p (j h one) -> p j h one", h=HPC, one=1
                ).broadcast_to([P, JT, HPC, D])
                nc.vector.tensor_tensor(
                    ot[:].rearrange("p (j h d) -> p j h d", h=HPC, d=D),
                    vview.rearrange("p j (h d) -> p j h d", d=D),
                    sctb,
                    op=Alu.mult,
                )
                nc.scalar.dma_start(o_d[:, ch * QW:(ch + 1) * QW], ot[:])

            for ch in range(NCH):
                b_front(ch)
                if ch >= 1:
                    b_back(ch - 1)
            b_back(NCH - 1)

    nc.compile()
    return nc


def get_nc():
    if "nc" not in _CACHE:
        _CACHE["nc"] = _build_nc()
    return _CACHE["nc"]


def shard_inputs(q, k, v):
    """core c -> (b = c//4, heads 2*(c%4), 2*(c%4)+1)."""
    maps = []
    for c in range(8):
        b, hp = divmod(c, 4)
        hs = slice(2 * hp, 2 * hp + 2)
        qs = q[b, :, hs, :].reshape(T, C).astype(np.float16)
        ks = k[b, :, hs, :].reshape(T, C).astype(np.float16)
        vs = v[b, :, hs, :].reshape(T, C).astype(np.float16)
        qt = np.ascontiguousarray(qs.T)                       # [128, 2048]
        kv = np.concatenate(
            [ks.reshape(NT, P, C), vs.reshape(NT, P, C)], axis=2
        )                                                     # [16, 128, 256]
        kvt = np.ascontiguousarray(
            kv.transpose(1, 0, 2).reshape(P, NT * KVW)
        )                                                     # [128, 4096]
        maps.append({"qt": qt, "kv": kvt})
    return maps


def gather_outputs(results):
    out = np.empty((B, T, H, D), dtype=np.float32)
    for c in range(8):
        b, hp = divmod(c, 4)
        ob = np.asarray(results[c]["o"])                      # [128, 2048] f16
        ot = ob.reshape(P, NT, C).transpose(1, 0, 2).reshape(T, HPC, D)
        out[b, :, 2 * hp:2 * hp + 2, :] = ot.astype(np.float32)
    return out


def kernel(q, k, v):
    from concourse.bass_utils import run_bass_kernel_spmd

    q = np.asarray(q, dtype=np.float32)
    k = np.asarray(k, dtype=np.float32)
    v = np.asarray(v, dtype=np.float32)
    nc = get_nc()
    maps = shard_inputs(q, k, v)
    res = run_bass_kernel_spmd(nc, maps, list(range(8)))
    return gather_outputs(res.results)


# revision 49
# speedup vs baseline: 1.0537x; 1.0537x over previous
"""Trainium2 Bass kernel for nn_CausalLinearSelfAttention_30013231464545.

Math: the reference cumsums the [B,T,H,D,M] kv tensor over axis=-2 (the D
axis), so the module collapses to
  out[b,t,h,m] = v[b,t,h,m] * s[b,t,h] / denom[b,t,h]
  denom = sum_d qf * cumsum_T(kf)   (causal running key sum)
  s     = sum_d qf * cumsum_D(kf)   (per-timestep D-prefix sum)
  qf = elu(q)+1 = min(exp(q),1) + relu(q),  kf likewise.

Sharding: core c -> (b = c//4, heads 2*(c%4)..2*(c%4)+1), slice [T=2048, 128].

v2 design (f16 end-to-end on chip, engine-balanced):
- Host casts to f16 and uploads: qT [128=(h,d), 2048=t] pre-transposed, and a
  pre-tiled kv blob [128=t-in-tile, (16 tiles x (k128|v128))]. Output is a
  packed [128, (16 tiles x 128)] f16 blob unpacked on host. All DMAs are
  128 descriptors x >=1KB (vs 512B in v1), input traffic halves.
- Feature maps: tensor_scalar relu/min clamps run in DVE 4x mode, one ACT Exp
  pass per tensor, one 2x tensor_tensor add.
- cumsum over T is computed TRANSPOSED on the PE: per t-tile, stationary =
  kf_j [t', (h,d)], moving = [triu128 | ones] so one matmul yields both the
  within-tile cumsum lcsT[(h,d), t] and the tile colsum (129th column).
  Cross-tile carries: tiny DVE prefix scan over colsums; the carry is added
  per-partition via the scalar slot of the denom-product scalar_tensor_tensor.
- cumsum over D: kfT via per-tile DMA transposes, then one block-triangular
  matmul per chunk -> csD_T in PSUM (replaces the slow DVE segmented scan).
- s/denom dots: DVE multiplies into a packed prods tile, then PE reduces over
  d-partitions per (tile, head) via a 0/1 selector matmul into sd_ps [8, 256];
  the cross-tile carry enters as bases.qf matmuls accumulated into the same
  PSUM tile.
- scale = s * recip(denom) (fast approx), PE-transposed back to [t, (j,h)],
  broadcast-multiplied into v, stored per chunk.
- Scheduling: engines dispatch in program order, so emission is two-phase
  (all features/cumsums, then dots/scale/output) and phase B is software-
  pipelined one chunk deep so the in-order DVE never head-of-line stalls
  on the PE's sd reduction.
"""

import numpy as np
import sys

sys.path.insert(0, "/opt/trn_rl_repo")

B, T, H, D = 2, 2048, 8, 64
P = 128          # partitions
HPC = 2          # heads per core
C = HPC * D      # 128: (h,d) width per core
NT = T // P      # 16 t-tiles
NCH = 4          # pipeline chunks
JT = NT // NCH   # 4 tiles per chunk
KVW = 2 * C      # 256: k|v columns per tile in the kv blob

DN_ON_POOL = False  # gpsimd cannot read PSUM on trn2 (BIR verifier)
DEBUG_SD = False    # dump per-chunk sd [8,256] (s|denom reductions) to DRAM

_CACHE = {}


def _build_nc():
    import concourse.bass as bass
    import concourse.bacc as bacc
    import concourse.mybir as mybir
    from concourse import tile

    dt = mybir.dt
    f32 = dt.float32
    f16 = dt.float16
    Alu = mybir.AluOpType
    Act = mybir.ActivationFunctionType

    nc = bacc.Bacc(None)

    qt_d = nc.declare_dram_parameter("qt", [P, T], f16, isOutput=False)
    kv_d = nc.declare_dram_parameter("kv", [P, NT * KVW], f16, isOutput=False)
    o_d = nc.declare_dram_parameter("o", [P, NT * C], f16, isOutput=True)
    if DEBUG_SD:
        dbg_d = nc.declare_dram_parameter("dbg", [8 * NCH, 256], f32,
                                          isOutput=True)

    # ---- constants, packed into one DRAM tensor [128, 296] ----
    # cols 0:128   tri: triu128 (within-tile cumsum; col 127 = colsum)
    # cols 128:256 btri: per-head upper-tri 64 blocks (cumsum over D)
    # cols 256:288 sel: (tile,head) selector columns for the d-reduce
    # cols 288:296 id8 (rows 0:8): identity for the scale transpose
    CW = 128 + 128 + 32 + 512
    cst = np.zeros((P, CW), dtype=np.float16)
    cst[:, 0:128] = np.triu(np.ones((P, P)))
    cst[:, 128:256] = np.kron(np.eye(2), np.triu(np.ones((64, 64))))
    for jl in range(JT):
        for i in range(8):
            # sel[p, jl*8+i] = 1 iff i == jl*2 + head(p)
            h = i - 2 * jl
            if h in (0, 1):
                cst[h * 64:(h + 1) * 64, 256 + jl * 8 + i] = 1.0
    cst[0:8, 288:296] = np.eye(8)
    cst_d = nc.inline_tensor(cst, name="cst_const")

    with tile.TileContext(nc) as tc:
        with (
            tc.tile_pool(name="const", bufs=1) as cpool,
            tc.tile_pool(name="io", bufs=1) as io,
            tc.tile_pool(name="ot", bufs=2) as otp,
            tc.tile_pool(name="wk", bufs=4) as wk,
            tc.tile_pool(name="sm", bufs=1) as sm,
            tc.tile_pool(name="pl", bufs=3, space="PSUM") as pl,
            tc.tile_pool(name="pc", bufs=1, space="PSUM") as pcd,
            tc.tile_pool(name="pd", bufs=2, space="PSUM") as pd,
            tc.tile_pool(name="pt", bufs=2, space="PSUM") as pt,
        ):
            qtw = io.tile([P, T], f16, tag="qt")
            kvw = io.tile([P, NT * KVW], f16, tag="kv")

            def ld(eng, dst, src, c0, c1, w):
                eng.dma_start(dst[:, c0 * w:(c1 + 1) * w],
                              src[:, c0 * w:(c1 + 1) * w])

            QW = JT * C        # 512 qT cols per chunk
            VW = JT * KVW      # 1024 kv cols per chunk
            # warm-up: a tiny sync-ring DMA plus an ACT->DVE handoff so the
            # first real crossings don't pay the cold semaphore latency
            warm = cpool.tile([1, 16], f16, tag="warm")
            nc.sync.dma_start(warm[:, 0:8], cst_d[0:1, 0:8])
            nc.scalar.activation(warm[:, 8:12], warm[:, 0:4], Act.Exp)
            nc.vector.tensor_scalar(warm[:, 12:16], warm[:, 8:12], 0.0, None,
                                    op0=Alu.max)
            ld(nc.sync, kvw, kv_d, 0, 0, VW)
            ld(nc.sync, qtw, qt_d, 0, 0, QW)
            ld(nc.sync, kvw, kv_d, 1, 1, VW)
            ld(nc.sync, qtw, qt_d, 1, 1, QW)
            ld(nc.sync, kvw, kv_d, 2, 3, VW)
            ld(nc.sync, qtw, qt_d, 2, 3, QW)

            cw = cpool.tile([P, CW], f16, tag="cst")
            nc.scalar.dma_start(cw[:], cst_d[:])
            tri = cw[:, 0:128]
            btri = cw[:, 128:256]
            sel = cw[:, 256:288]
            id8 = cw[0:8, 288:296]

            kfw = sm.tile([P, T], f16, tag="kfw")        # feature(k), full
            kftw = sm.tile([P, T], f16, tag="kftw")      # kf transposed
            sctc = sm.tile([P, 8 * NCH], f16, tag="sctc")  # scale, SBUF copy
            cs_sb = sm.tile([P, NT], f16, tag="cs")      # tile colsums
            btmp = sm.tile([P, NT], f16, tag="btmp")     # inclusive prefix
            bases = sm.tile([P, NT], f16, tag="bases")   # exclusive prefix
            ones = sm.tile([P, NT], f16, tag="ones")
            nc.gpsimd.memset(ones[:], 1.0)

            qfts, lcss, csds = [], [], []

            # ---------- phase A: features, cumsums, carries ----------
            for ch in range(NCH):
                j0 = ch * JT
                kview = kvw[:, ch * VW:(ch + 1) * VW].rearrange(
                    "p (j c) -> p j c", c=KVW)[:, :, 0:C]
                qsl = qtw[:, ch * QW:(ch + 1) * QW]

                # feature maps: f = min(exp(x),1) + relu(x)
                kf = kfw[:, ch * QW:(ch + 1) * QW]
                rk = wk.tile([P, QW], f16, tag="rk")
                nc.vector.tensor_scalar(
                    rk[:].rearrange("p (j c) -> p j c", c=C),
                    kview, 0.0, None, op0=Alu.max)
                ek = wk.tile([P, QW], f16, tag="ek")
                nc.scalar.activation(
                    ek[:].rearrange("p (j c) -> p j c", c=C),
                    kview, Act.Exp)
                emk = wk.tile([P, QW], f16, tag="emk")
                nc.vector.tensor_scalar(emk[:], ek[:], 1.0, None, op0=Alu.min)
                nc.vector.tensor_tensor(kf, emk[:], rk[:], op=Alu.add)

                rq = wk.tile([P, QW], f16, tag="rq")
                nc.vector.tensor_scalar(rq[:], qsl, 0.0, None, op0=Alu.max)
                eq = wk.tile([P, QW], f16, tag="eq")
                nc.scalar.activation(eq[:], qsl, Act.Exp)
                emq = wk.tile([P, QW], f16, tag="emq")
                nc.vector.tensor_scalar(emq[:], eq[:], 1.0, None, op0=Alu.min)
                qft = wk.tile([P, QW], f16, tag="qft")
                nc.vector.tensor_tensor(qft[:], emq[:], rq[:], op=Alu.add)
                qfts.append(qft)

                # transposed within-tile cumsum over T (PE); the cumsum's
                # last column doubles as the tile colsum
                lcs = pl.tile([P, JT * C], f32, tag="lcs")
                for jl in range(JT):
                    nc.tensor.matmul(
                        lcs[:, jl * C:(jl + 1) * C],
                        kfw[:, ch * QW + jl * C: ch * QW + (jl + 1) * C],
                        tri, start=True, stop=True,
                    )
                lcss.append(lcs)

                # kfT via batched per-block DMA transpose, then csD (PE)
                kft = kftw[:, ch * QW:(ch + 1) * QW]
                nc.sync.dma_start_transpose(
                    kft.rearrange("p (j t) -> p j t", t=P),
                    kfw[:, ch * QW:(ch + 1) * QW])
                csd = pcd.tile([P, QW], f32, tag="csd")
                nc.tensor.matmul(csd[:], btri, kft, start=True, stop=True)
                csds.append(csd)

            # ---------- phase B: dots, reduce, scale, output ----------
            # Software-pipelined one chunk deep: chunk ch+1's products are
            # emitted before chunk ch's scale/output so the in-order DVE
            # never head-of-line stalls on the PE's sd reduction.
            sds = [None] * NCH

            def b_front(ch):
                j0 = ch * JT
                qft, lcs, csd = qfts[ch], lcss[ch], csds[ch]

                # colsum extraction + cross-tile carry prefix
                nc.scalar.copy(
                    cs_sb[:, j0:j0 + JT].rearrange("p (j o) -> p j o", o=1),
                    lcs[:].rearrange("p (j c) -> p j c", c=C)[:, :, 127:128],
                )
                w = j0 + JT
                nc.vector.tensor_tensor_scan(
                    btmp[:, 0:w], ones[:, 0:w], cs_sb[:, 0:w], 0.0,
                    op0=Alu.mult, op1=Alu.add,
                )
                nc.vector.tensor_tensor(
                    bases[:, j0:w], btmp[:, j0:w], cs_sb[:, j0:w],
                    op=Alu.subtract,
                )

                # copy cumsums PSUM -> SBUF f16 on the idle scalar engine
                # so both product multiplies run in DVE 2x mode
                lcsb = wk.tile([P, QW], f16, tag="lcsb")
                nc.scalar.copy(lcsb[:], lcs[:])
                csdb = wk.tile([P, QW], f16, tag="csdb")
                nc.scalar.copy(csdb[:], csd[:])
                lsrc, csrc = lcsb[:], csdb[:]

                # prods = [s-prod | dn-prod] per tile; the cross-tile carry
                # enters below as extra PE matmuls on the dn columns
                prods = wk.tile([P, JT * 256], f16, tag="prods")
                pview = prods[:].rearrange("p (j c) -> p j c", c=256)
                nc.vector.tensor_tensor(
                    pview[:, :, 128:256],
                    lsrc.rearrange("p (j c) -> p j c", c=C),
                    qft[:].rearrange("p (j c) -> p j c", c=C),
                    op=Alu.mult,
                )
                nc.vector.tensor_tensor(
                    pview[:, :, 0:128],
                    csrc.rearrange("p (j c) -> p j c", c=C),
                    qft[:].rearrange("p (j c) -> p j c", c=C),
                    op=Alu.mult,
                )
                # basesM[p, (jl,i)] = bases[p, j0+jl] * sel[p, (jl,i)]
                basm = wk.tile([P, JT * 8], f16, tag="basm")
                nc.vector.tensor_tensor(
                    basm[:].rearrange("p (j i) -> p j i", i=8),
                    bases[:, j0:j0 + JT].rearrange(
                        "p (j o) -> p j o", o=1).broadcast_to([P, JT, 8]),
                    sel.rearrange("p (j i) -> p j i", i=8),
                    op=Alu.mult,
                )

                # d-reduce per (tile, head) on PE -> sd [8, 256]
                sd = pd.tile([8, 256], f32, tag="sd")
                for jl in range(JT):
                    nc.tensor.matmul(
                        sd[:],
                        sel[:, jl * 8:(jl + 1) * 8],
                        prods[:, jl * 256:(jl + 1) * 256],
                        start=(jl == 0), stop=False,
                    )
                for jl in range(JT):
                    nc.tensor.matmul(
                        sd[:, 128:256],
                        basm[:, jl * 8:(jl + 1) * 8],
                        qft[:, jl * C:(jl + 1) * C],
                        start=False, stop=(jl == JT - 1),
                    )
                sds[ch] = sd

            def b_back(ch):
                sd = sds[ch]
                vview = kvw[:, ch * VW:(ch + 1) * VW].rearrange(
                    "p (j c) -> p j c", c=KVW)[:, :, C:KVW]
                if DEBUG_SD:
                    sdc = sm.tile([8, 256], f32, tag=f"sdc{ch}")
                    nc.scalar.copy(sdc[:], sd[:])
                    nc.sync.dma_start(dbg_d[ch * 8:(ch + 1) * 8, :], sdc[:])

                # scale = s / denom, transposed back to [t, (j,h)]
                rec = sm.tile([8, P], f32, tag=f"rec{ch}")
                nc.vector.reciprocal_approx_fast(rec[:], sd[:, 128:256])
                ssm = sm.tile([8, P], f16, tag=f"ssm{ch}")
                nc.vector.tensor_tensor(ssm[:], sd[:, 0:128], rec[:],
                                        op=Alu.mult)
                sct = pt.tile([P, 8], f16, tag="sct")
                nc.tensor.transpose(sct[:], ssm[:], id8)

                # out = v * scale (broadcast over each head's 64 columns)
                ot = otp.tile([P, JT * C], f16, tag="o")
                sctb = sct[:].rearrange(
                    "p (j h one) -> p j h one", h=HPC, one=1
                ).broadcast_to([P, JT, HPC, D])
                nc.vector.tensor_tensor(
                    ot[:].rearrange("p (j h d) -> p j h d", h=HPC, d=D),
                    vview.rearrange("p j (h d) -> p j h d", d=D),
                    sctb,
                    op=Alu.mult,
                )
                nc.scalar.dma_start(o_d[:, ch * QW:(ch + 1) * QW], ot[:])

            for ch in range(NCH):
                b_front(ch)
                if ch >= 1:
                    b_back(ch - 1)
            b_back(NCH - 1)

    nc.compile()
    return nc


def get_nc():
    if "nc" not in _CACHE:
        _CACHE["nc"] = _build_nc()
    return _CACHE["nc"]


def shard_inputs(q, k, v):
    """core c -> (b = c//4, heads 2*(c%4), 2*(c%4)+1)."""
    maps = []
    for c in range(8):
        b, hp = divmod(c, 4)
        hs = slice(2 * hp, 2 * hp + 2)
        qs = q[b, :, hs, :].reshape(T, C).astype(np.float16)
        ks = k[b, :, hs, :].reshape(T, C).astype(np.float16)
        vs = v[b, :, hs, :].reshape(T, C).astype(np.float16)
        qt = np.ascontiguousarray(qs.T)                       # [128, 2048]
        kv = np.concatenate(
            [ks.reshape(NT, P, C), vs.reshape(NT, P, C)], axis=2
        )                                                     # [16, 128, 256]
        kvt = np.ascontiguousarray(
            kv.transpose(1, 0, 2).reshape(P, NT * KVW)
        )                                                     # [128, 4096]
        maps.append({"qt": qt, "kv": kvt})
    return maps


def gather_outputs(results):
    out = np.empty((B, T, H, D), dtype=np.float32)
    for c in range(8):
        b, hp = divmod(c, 4)
        ob = np.asarray(results[c]["o"])                      # [128, 2048] f16
        ot = ob.reshape(P, NT, C).transpose(1, 0, 2).reshape(T, HPC, D)
        out[b, :, 2 * hp:2 * hp + 2, :] = ot.astype(np.float32)
    return out


def kernel(q, k, v):
    from concourse.bass_utils import run_bass_kernel_spmd

    q = np.asarray(q, dtype=np.float32)
    k = np.asarray(k, dtype=np.float32)
    v = np.asarray(v, dtype=np.float32)
    nc = get_nc()
    maps = shard_inputs(q, k, v)
    res = run_bass_kernel_spmd(nc, maps, list(range(8)))
    return gather_outputs(res.results)


# revision 50
# speedup vs baseline: 1.0676x; 1.0132x over previous
"""Trainium2 Bass kernel for nn_CausalLinearSelfAttention_30013231464545.

Math: the reference cumsums the [B,T,H,D,M] kv tensor over axis=-2 (the D
axis), so the module collapses to
  out[b,t,h,m] = v[b,t,h,m] * s[b,t,h] / denom[b,t,h]
  denom = sum_d qf * cumsum_T(kf)   (causal running key sum)
  s     = sum_d qf * cumsum_D(kf)   (per-timestep D-prefix sum)
  qf = elu(q)+1 = min(exp(q),1) + relu(q),  kf likewise.

Sharding: core c -> (b = c//4, heads 2*(c%4)..2*(c%4)+1), slice [T=2048, 128].

v2 design (f16 end-to-end on chip, engine-balanced):
- Host casts to f16 and uploads: qT [128=(h,d), 2048=t] pre-transposed, and a
  pre-tiled kv blob [128=t-in-tile, (16 tiles x (k128|v128))]. Output is a
  packed [128, (16 tiles x 128)] f16 blob unpacked on host. All DMAs are
  128 descriptors x >=1KB (vs 512B in v1), input traffic halves.
- Feature maps: tensor_scalar relu/min clamps run in DVE 4x mode, one ACT Exp
  pass per tensor, one 2x tensor_tensor add.
- cumsum over T is computed TRANSPOSED on the PE: per t-tile, stationary =
  kf_j [t', (h,d)], moving = [triu128 | ones] so one matmul yields both the
  within-tile cumsum lcsT[(h,d), t] and the tile colsum (129th column).
  Cross-tile carries: tiny DVE prefix scan over colsums; the carry is added
  per-partition via the scalar slot of the denom-product scalar_tensor_tensor.
- cumsum over D: kfT via per-tile DMA transposes, then one block-triangular
  matmul per chunk -> csD_T in PSUM (replaces the slow DVE segmented scan).
- s/denom dots: DVE multiplies into a packed prods tile, then PE reduces over
  d-partitions per (tile, head) via a 0/1 selector matmul into sd_ps [8, 256];
  the cross-tile carry enters as bases.qf matmuls accumulated into the same
  PSUM tile.
- scale = s * recip(denom) (fast approx), PE-transposed back to [t, (j,h)],
  broadcast-multiplied into v, stored per chunk.
- Scheduling: engines dispatch in program order, so emission is two-phase
  (all features/cumsums, then dots/scale/output) and phase B is software-
  pipelined one chunk deep so the in-order DVE never head-of-line stalls
  on the PE's sd reduction.
"""

import numpy as np
import sys

sys.path.insert(0, "/opt/trn_rl_repo")

B, T, H, D = 2, 2048, 8, 64
P = 128          # partitions
HPC = 2          # heads per core
C = HPC * D      # 128: (h,d) width per core
NT = T // P      # 16 t-tiles
NCH = 4          # pipeline chunks
JT = NT // NCH   # 4 tiles per chunk
KVW = 2 * C      # 256: k|v columns per tile in the kv blob

DN_ON_POOL = False  # gpsimd cannot read PSUM on trn2 (BIR verifier)
DEBUG_SD = False    # dump per-chunk sd [8,256] (s|denom reductions) to DRAM

_CACHE = {}


def _build_nc():
    import concourse.bass as bass
    import concourse.bacc as bacc
    import concourse.mybir as mybir
    from concourse import tile

    dt = mybir.dt
    f32 = dt.float32
    f16 = dt.float16
    Alu = mybir.AluOpType
    Act = mybir.ActivationFunctionType

    nc = bacc.Bacc(None)

    qt_d = nc.declare_dram_parameter("qt", [P, T], f16, isOutput=False)
    kv_d = nc.declare_dram_parameter("kv", [P, NT * KVW], f16, isOutput=False)
    o_d = nc.declare_dram_parameter("o", [P, NT * C], f16, isOutput=True)
    if DEBUG_SD:
        dbg_d = nc.declare_dram_parameter("dbg", [8 * NCH, 256], f32,
                                          isOutput=True)

    # ---- constants, packed into one DRAM tensor [128, 296] ----
    # cols 0:128   tri: triu128 (within-tile cumsum; col 127 = colsum)
    # cols 128:256 btri: per-head upper-tri 64 blocks (cumsum over D)
    # cols 256:288 sel: (tile,head) selector columns for the d-reduce
    # cols 288:296 id8 (rows 0:8): identity for the scale transpose
    CW = 128 + 128 + 32 + 512
    cst = np.zeros((P, CW), dtype=np.float16)
    cst[:, 0:128] = np.triu(np.ones((P, P)))
    cst[:, 128:256] = np.kron(np.eye(2), np.triu(np.ones((64, 64))))
    for jl in range(JT):
        for i in range(8):
            # sel[p, jl*8+i] = 1 iff i == jl*2 + head(p)
            h = i - 2 * jl
            if h in (0, 1):
                cst[h * 64:(h + 1) * 64, 256 + jl * 8 + i] = 1.0
    cst[0:8, 288:296] = np.eye(8)
    cst_d = nc.inline_tensor(cst, name="cst_const")

    with tile.TileContext(nc) as tc:
        with (
            tc.tile_pool(name="const", bufs=1) as cpool,
            tc.tile_pool(name="io", bufs=1) as io,
            tc.tile_pool(name="ot", bufs=2) as otp,
            tc.tile_pool(name="wk", bufs=4) as wk,
            tc.tile_pool(name="sm", bufs=1) as sm,
            tc.tile_pool(name="pl", bufs=3, space="PSUM") as pl,
            tc.tile_pool(name="pc", bufs=1, space="PSUM") as pcd,
            tc.tile_pool(name="pd", bufs=2, space="PSUM") as pd,
            tc.tile_pool(name="pt", bufs=2, space="PSUM") as pt,
        ):
            qtw = io.tile([P, T], f16, tag="qt")
            kvw = io.tile([P, NT * KVW], f16, tag="kv")

            def ld(eng, dst, src, c0, c1, w):
                eng.dma_start(dst[:, c0 * w:(c1 + 1) * w],
                              src[:, c0 * w:(c1 + 1) * w])

            QW = JT * C        # 512 qT cols per chunk
            VW = JT * KVW      # 1024 kv cols per chunk
            # warm-up: a tiny sync-ring DMA plus an ACT->DVE handoff so the
            # first real crossings don't pay the cold semaphore latency
            warm = cpool.tile([1, 16], f16, tag="warm")
            nc.sync.dma_start(warm[:, 0:8], cst_d[0:1, 0:8])
            nc.scalar.activation(warm[:, 8:12], warm[:, 0:4], Act.Exp)
            nc.vector.tensor_scalar(warm[:, 12:16], warm[:, 8:12], 0.0, None,
                                    op0=Alu.max)
            ld(nc.sync, kvw, kv_d, 0, 0, VW)
            ld(nc.sync, qtw, qt_d, 0, 0, QW)
            ld(nc.sync, kvw, kv_d, 1, 1, VW)
            ld(nc.sync, qtw, qt_d, 1, 1, QW)
            ld(nc.sync, kvw, kv_d, 2, 3, VW)
            ld(nc.sync, qtw, qt_d, 2, 3, QW)

            cw = cpool.tile([P, CW], f16, tag="cst")
            nc.scalar.dma_start(cw[:], cst_d[:])
            tri = cw[:, 0:128]
            btri = cw[:, 128:256]
            sel = cw[:, 256:288]
            id8 = cw[0:8, 288:296]

            kfw = sm.tile([P, T], f16, tag="kfw")        # feature(k), full
            kftw = sm.tile([P, T], f16, tag="kftw")      # kf transposed
            sctc = sm.tile([P, 8 * NCH], f16, tag="sctc")  # scale, SBUF copy
            cs_sb = sm.tile([P, NT], f16, tag="cs")      # tile colsums
            btmp = sm.tile([P, NT], f16, tag="btmp")     # inclusive prefix
            bases = sm.tile([P, NT], f16, tag="bases")   # exclusive prefix
            ones = sm.tile([P, NT], f16, tag="ones")
            nc.gpsimd.memset(ones[:], 1.0)

            qfts, lcss, csds = [], [], []

            # ---------- phase A1: k features + cumsum chain (the long
            # pole: kf -> lcs -> DMA transpose -> csD) for all chunks ----
            for ch in range(NCH):
                kview = kvw[:, ch * VW:(ch + 1) * VW].rearrange(
                    "p (j c) -> p j c", c=KVW)[:, :, 0:C]

                # feature maps: f = min(exp(x),1) + relu(x)
                kf = kfw[:, ch * QW:(ch + 1) * QW]
                rk = wk.tile([P, QW], f16, tag="rk")
                nc.vector.tensor_scalar(
                    rk[:].rearrange("p (j c) -> p j c", c=C),
                    kview, 0.0, None, op0=Alu.max)
                ek = wk.tile([P, QW], f16, tag="ek")
                nc.scalar.activation(
                    ek[:].rearrange("p (j c) -> p j c", c=C),
                    kview, Act.Exp)
                emk = wk.tile([P, QW], f16, tag="emk")
                nc.vector.tensor_scalar(emk[:], ek[:], 1.0, None, op0=Alu.min)
                nc.vector.tensor_tensor(kf, emk[:], rk[:], op=Alu.add)

                # transposed within-tile cumsum over T (PE); the cumsum's
                # last column doubles as the tile colsum
                lcs = pl.tile([P, JT * C], f32, tag="lcs")
                for jl in range(JT):
                    nc.tensor.matmul(
                        lcs[:, jl * C:(jl + 1) * C],
                        kfw[:, ch * QW + jl * C: ch * QW + (jl + 1) * C],
                        tri, start=True, stop=True,
                    )
                lcss.append(lcs)

                # kfT via batched per-block DMA transpose, then csD (PE)
                kft = kftw[:, ch * QW:(ch + 1) * QW]
                nc.sync.dma_start_transpose(
                    kft.rearrange("p (j t) -> p j t", t=P),
                    kfw[:, ch * QW:(ch + 1) * QW])
                csd = pcd.tile([P, QW], f32, tag="csd")
                nc.tensor.matmul(csd[:], btri, kft, start=True, stop=True)
                csds.append(csd)

            # ---------- phase A2: q features ----------
            for ch in range(NCH):
                qsl = qtw[:, ch * QW:(ch + 1) * QW]
                rq = wk.tile([P, QW], f16, tag="rq")
                nc.vector.tensor_scalar(rq[:], qsl, 0.0, None, op0=Alu.max)
                eq = wk.tile([P, QW], f16, tag="eq")
                nc.scalar.activation(eq[:], qsl, Act.Exp)
                emq = wk.tile([P, QW], f16, tag="emq")
                nc.vector.tensor_scalar(emq[:], eq[:], 1.0, None, op0=Alu.min)
                qft = wk.tile([P, QW], f16, tag="qft")
                nc.vector.tensor_tensor(qft[:], emq[:], rq[:], op=Alu.add)
                qfts.append(qft)

            # ---------- phase B: dots, reduce, scale, output ----------
            # Software-pipelined one chunk deep: chunk ch+1's products are
            # emitted before chunk ch's scale/output so the in-order DVE
            # never head-of-line stalls on the PE's sd reduction.
            sds = [None] * NCH

            def b_front(ch):
                j0 = ch * JT
                qft, lcs, csd = qfts[ch], lcss[ch], csds[ch]

                # colsum extraction + cross-tile carry prefix
                nc.scalar.copy(
                    cs_sb[:, j0:j0 + JT].rearrange("p (j o) -> p j o", o=1),
                    lcs[:].rearrange("p (j c) -> p j c", c=C)[:, :, 127:128],
                )
                w = j0 + JT
                nc.vector.tensor_tensor_scan(
                    btmp[:, 0:w], ones[:, 0:w], cs_sb[:, 0:w], 0.0,
                    op0=Alu.mult, op1=Alu.add,
                )
                nc.vector.tensor_tensor(
                    bases[:, j0:w], btmp[:, j0:w], cs_sb[:, j0:w],
                    op=Alu.subtract,
                )

                # copy cumsums PSUM -> SBUF f16 on the idle scalar engine
                # so both product multiplies run in DVE 2x mode
                lcsb = wk.tile([P, QW], f16, tag="lcsb")
                nc.scalar.copy(lcsb[:], lcs[:])
                csdb = wk.tile([P, QW], f16, tag="csdb")
                nc.scalar.copy(csdb[:], csd[:])
                lsrc, csrc = lcsb[:], csdb[:]

                # prods = [s-prod | dn-prod] per tile; the cross-tile carry
                # enters below as extra PE matmuls on the dn columns
                prods = wk.tile([P, JT * 256], f16, tag="prods")
                pview = prods[:].rearrange("p (j c) -> p j c", c=256)
                nc.vector.tensor_tensor(
                    pview[:, :, 128:256],
                    lsrc.rearrange("p (j c) -> p j c", c=C),
                    qft[:].rearrange("p (j c) -> p j c", c=C),
                    op=Alu.mult,
                )
                nc.vector.tensor_tensor(
                    pview[:, :, 0:128],
                    csrc.rearrange("p (j c) -> p j c", c=C),
                    qft[:].rearrange("p (j c) -> p j c", c=C),
                    op=Alu.mult,
                )
                # basesM[p, (jl,i)] = bases[p, j0+jl] * sel[p, (jl,i)]
                basm = wk.tile([P, JT * 8], f16, tag="basm")
                nc.vector.tensor_tensor(
                    basm[:].rearrange("p (j i) -> p j i", i=8),
                    bases[:, j0:j0 + JT].rearrange(
                        "p (j o) -> p j o", o=1).broadcast_to([P, JT, 8]),
                    sel.rearrange("p (j i) -> p j i", i=8),
                    op=Alu.mult,
                )

                # d-reduce per (tile, head) on PE -> sd [8, 256]
                sd = pd.tile([8, 256], f32, tag="sd")
                for jl in range(JT):
                    nc.tensor.matmul(
                        sd[:],
                        sel[:, jl * 8:(jl + 1) * 8],
                        prods[:, jl * 256:(jl + 1) * 256],
                        start=(jl == 0), stop=False,
                    )
                for jl in range(JT):
                    nc.tensor.matmul(
                        sd[:, 128:256],
                        basm[:, jl * 8:(jl + 1) * 8],
                        qft[:, jl * C:(jl + 1) * C],
                        start=False, stop=(jl == JT - 1),
                    )
                sds[ch] = sd

            def b_back(ch):
                sd = sds[ch]
                vview = kvw[:, ch * VW:(ch + 1) * VW].rearrange(
                    "p (j c) -> p j c", c=KVW)[:, :, C:KVW]
                if DEBUG_SD:
                    sdc = sm.tile([8, 256], f32, tag=f"sdc{ch}")
                    nc.scalar.copy(sdc[:], sd[:])
                    nc.sync.dma_start(dbg_d[ch * 8:(ch + 1) * 8, :], sdc[:])

                # scale = s / denom, transposed back to [t, (j,h)]
                rec = sm.tile([8, P], f32, tag=f"rec{ch}")
                nc.vector.reciprocal_approx_fast(rec[:], sd[:, 128:256])
                ssm = sm.tile([8, P], f16, tag=f"ssm{ch}")
                nc.vector.tensor_tensor(ssm[:], sd[:, 0:128], rec[:],
                                        op=Alu.mult)
                sct = pt.tile([P, 8], f16, tag="sct")
                nc.tensor.transpose(sct[:], ssm[:], id8)

                # out = v * scale (broadcast over each head's 64 columns)
                ot = otp.tile([P, JT * C], f16, tag="o")
                sctb = sct[:].rearrange(
                    "p (j h one) -> p j h one", h=HPC, one=1
                ).broadcast_to([P, JT, HPC, D])
                nc.vector.tensor_tensor(
                    ot[:].rearrange("p (j h d) -> p j h d", h=HPC, d=D),
                    vview.rearrange("p j (h d) -> p j h d", d=D),
                    sctb,
                    op=Alu.mult,
                )
                nc.scalar.dma_start(o_d[:, ch * QW:(ch + 1) * QW], ot[:])

            for ch in range(NCH):
                b_front(ch)
                if ch >= 1:
                    b_back(ch - 1)
            b_back(NCH - 1)

    nc.compile()
    return nc


def get_nc():
    if "nc" not in _CACHE:
        _CACHE["nc"] = _build_nc()
    return _CACHE["nc"]


def shard_inputs(q, k, v):
    """core c -> (b = c//4, heads 2*(c%4), 2*(c%4)+1)."""
    maps = []
    for c in range(8):
        b, hp = divmod(c, 4)
        hs = slice(2 * hp, 2 * hp + 2)
        qs = q[b, :, hs, :].reshape(T, C).astype(np.float16)
        ks = k[b, :, hs, :].reshape(T, C).astype(np.float16)
        vs = v[b, :, hs, :].reshape(T, C).astype(np.float16)
        qt = np.ascontiguousarray(qs.T)                       # [128, 2048]
        kv = np.concatenate(
            [ks.reshape(NT, P, C), vs.reshape(NT, P, C)], axis=2
        )                                                     # [16, 128, 256]
        kvt = np.ascontiguousarray(
            kv.transpose(1, 0, 2).reshape(P, NT * KVW)
        )                                                     # [128, 4096]
        maps.append({"qt": qt, "kv": kvt})
    return maps


def gather_outputs(results):
    out = np.empty((B, T, H, D), dtype=np.float32)
    for c in range(8):
        b, hp = divmod(c, 4)
        ob = np.asarray(results[c]["o"])                      # [128, 2048] f16
        ot = ob.reshape(P, NT, C).transpose(1, 0, 2).reshape(T, HPC, D)
        out[b, :, 2 * hp:2 * hp + 2, :] = ot.astype(np.float32)
    return out


def kernel(q, k, v):
    from concourse.bass_utils import run_bass_kernel_spmd

    q = np.asarray(q, dtype=np.float32)
    k = np.asarray(k, dtype=np.float32)
    v = np.asarray(v, dtype=np.float32)
    nc = get_nc()
    maps = shard_inputs(q, k, v)
    res = run_bass_kernel_spmd(nc, maps, list(range(8)))
    return gather_outputs(res.results)
